# revision 1
# baseline (speedup 1.0000x reference)
"""LoRA layer kernel for Trainium2, SPMD across 8 NeuronCores.

Computes: out[b,s,h,d] = x[b,s,:] @ W_orig[:,h,d] + SCALE * (x @ A) @ B[:,h,d]

Strategy (per sharding hint, data-parallel branch):
  - Fold LoRA into the weights ON DEVICE: W_eff = W + (SCALE*A) @ B
    (associativity of matmul makes this exact up to fp rounding, and it
    turns the whole problem into one dense matmul).
  - Shard x over tokens (B*S = 8192 -> 1024 tokens per core); W/A/B replicated.
  - Per core: out_slice[1024, 2048] = xT_slice.T @ W_eff, accumulated over
    16 K-tiles of 128 into 4 PSUM banks of [128, 512].
  - Matmuls run in float32r mode (fp32 bits, FP22 multiply, fp32 accumulate):
    4x faster than true fp32 on the PE at ~1e-4 relative error.

x is fed pre-transposed ([H, tokens] per core) so the contraction dim lands on
SBUF partitions; host-side layout prep only, all FLOPs happen on device.
"""

import numpy as np

# Problem shapes (hardcoded per contract - kernel.py must be self-contained)
B, S, H = 4, 2048, 2048
NH, HD = 16, 128
N = NH * HD            # 2048 output features
RANK = 4
ALPHA = 4.0
SCALE = ALPHA / RANK   # 1.0
NCORES = 8
TOK = B * S            # 8192 tokens total
TPC = TOK // NCORES    # 1024 tokens per core

P = 128                # SBUF partitions
KT = H // P            # 16 contraction tiles
TT = TPC // P          # 8 token tiles per core
CH = 512               # psum chunk width (fp32 moving-operand / bank limit)
NCH = N // CH          # 4 chunks

_CACHE = {}


def _build_program(reps=1):
    """Build the SPMD program. reps>1 repeats the whole body back-to-back
    (used only for timing: wall(R) - wall(1) cancels host/tunnel overhead)."""
    import concourse.mybir as mybir
    import concourse.tile as tile
    from concourse import bacc

    f32 = mybir.dt.float32
    f32r = mybir.dt.float32r
    bf16 = mybir.dt.bfloat16

    nc = bacc.Bacc(None, target_bir_lowering=False, debug=False)

    # Main matmul runs in bf16 (inputs rounded on device; fp32 PSUM
    # accumulation). The LoRA A@B fold runs in float32r (fp32 bits, FP22
    # multiply) and its DVE add writes the resident W_eff tiles as bf16.
    xt = nc.dram_tensor("xt", [H, TPC], f32, kind="ExternalInput")
    w = nc.dram_tensor("w", [H, N], f32, kind="ExternalInput")
    at = nc.dram_tensor("at", [RANK, H], f32r, kind="ExternalInput")
    bk = nc.dram_tensor("bk", [RANK, N], f32r, kind="ExternalInput")
    out = nc.dram_tensor("out", [TPC, N], f32, kind="ExternalOutput")

    with tile.TileContext(nc) as tc:
        with (
            tc.tile_pool(name="wpool", bufs=1) as wpool,
            tc.tile_pool(name="wraw", bufs=3) as wraw,
            tc.tile_pool(name="xpool", bufs=3) as xpool,
            tc.tile_pool(name="opool", bufs=2) as opool,
            tc.tile_pool(name="cpool", bufs=1) as cpool,
            tc.tile_pool(name="apool", bufs=2) as apool,
            tc.tile_pool(name="psum", bufs=8, space="PSUM") as psum,
        ):
            for r in range(reps):
                # LoRA B matrix, resident: [RANK, N]
                bk_t = cpool.tile([RANK, N], f32r, tag="bk", name=f"bk_{r}")
                nc.sync.dma_start(bk_t[:], bk[:])

                # Prefetch first token tiles of x while W streams in.
                # Each x tile holds a full [H, 128-token] slab as [p, k, t];
                # ScalarE (otherwise idle) downcasts fp32 -> bf16.
                x_tiles = {}

                def load_x(t, r=r):
                    xr = xpool.tile([P, KT, P], f32, tag="xr",
                                    name=f"xr_{r}_{t}")
                    src = xt[:, t * P:(t + 1) * P].rearrange(
                        "(k p) t -> p k t", p=P)
                    nc.sync.dma_start(xr[:], src)
                    x3 = xpool.tile([P, KT, P], bf16, tag="x",
                                    name=f"x3_{r}_{t}")
                    nc.scalar.copy(x3[:], xr[:])
                    x_tiles[t] = x3

                load_x(0)
                load_x(1)

                # Phase 1: stream W in, fold LoRA: W_eff[k] = W[k] + A_k @ B.
                # The DVE add reads raw W (fp32) + lora product (PSUM fp32)
                # and writes the resident W_eff tile as f32r (rounds on
                # output).
                w_tiles = []
                for k in range(KT):
                    wr = wraw.tile([P, N], f32, tag="wr", name=f"wr_{r}_{k}")
                    nc.sync.dma_start(wr[:], w[k * P:(k + 1) * P, :])
                    wt = wpool.tile([P, N], bf16, tag=f"w{k}",
                                    name=f"weff_{r}_{k}")
                    at_t = apool.tile([RANK, P], f32r, tag="at",
                                      name=f"at_{r}_{k}")
                    nc.sync.dma_start(at_t[:], at[:, k * P:(k + 1) * P])
                    for c in range(NCH):
                        ps = psum.tile([P, CH], f32, tag="ps",
                                       name=f"psl_{r}_{k}_{c}")
                        nc.tensor.matmul(
                            ps[:],
                            at_t[:],
                            bk_t[:, c * CH:(c + 1) * CH],
                            start=True, stop=True,
                        )
                        nc.vector.tensor_add(
                            wt[:, c * CH:(c + 1) * CH],
                            wr[:, c * CH:(c + 1) * CH],
                            ps[:],
                        )
                    w_tiles.append(wt)

                # Phase 2: main matmul, token tile by token tile
                for t in range(TT):
                    x3 = x_tiles.pop(t)
                    if t + 2 < TT:
                        load_x(t + 2)
                    pss = [
                        psum.tile([P, CH], f32, tag="ps", name=f"ps_{r}_{t}_{c}")
                        for c in range(NCH)
                    ]
                    for k in range(KT):
                        lhsT = x3[:, k, :]
                        for c in range(NCH):
                            nc.tensor.matmul(
                                pss[c][:],
                                lhsT,
                                w_tiles[k][:, c * CH:(c + 1) * CH],
                                start=(k == 0), stop=(k == KT - 1),
                            )
                    ot = opool.tile([P, N], f32, tag="o", name=f"o_{r}_{t}")
                    for c in range(NCH):
                        nc.vector.tensor_copy(ot[:, c * CH:(c + 1) * CH],
                                              pss[c][:])
                    nc.sync.dma_start(out[t * P:(t + 1) * P, :], ot[:])

    nc.compile()
    return nc


def _prep_inputs(x, W_orig, A_kernel, B_kernel):
    x = np.asarray(x, dtype=np.float32)
    W_orig = np.asarray(W_orig, dtype=np.float32)
    A_kernel = np.asarray(A_kernel, dtype=np.float32)
    B_kernel = np.asarray(B_kernel, dtype=np.float32)

    xT = np.ascontiguousarray(x.reshape(TOK, H).T)          # [H, TOK]
    w2d = np.ascontiguousarray(W_orig.reshape(H, N))        # [H, N]
    at = np.ascontiguousarray(A_kernel.T) * np.float32(SCALE)  # [RANK, H]
    bk = np.ascontiguousarray(B_kernel.reshape(RANK, N))    # [RANK, N]

    in_maps = []
    for i in range(NCORES):
        in_maps.append({
            "xt": np.ascontiguousarray(xT[:, i * TPC:(i + 1) * TPC]),
            "w": w2d,
            "at": at,
            "bk": bk,
        })
    return in_maps


def kernel(x, W_orig, A_kernel, B_kernel):
    from concourse.bass_utils import run_bass_kernel_spmd

    if "nc" not in _CACHE:
        _CACHE["nc"] = _build_program()
    nc = _CACHE["nc"]

    in_maps = _prep_inputs(x, W_orig, A_kernel, B_kernel)
    res = run_bass_kernel_spmd(nc, in_maps, list(range(NCORES)))
    parts = [res.results[i]["out"] for i in range(NCORES)]
    full = np.concatenate(parts, axis=0)                    # [TOK, N]
    return full.reshape(B, S, NH, HD)



# revision 3
# speedup vs baseline: 1.2403x; 1.2403x over previous
"""LoRA layer kernel for Trainium2, SPMD across 8 NeuronCores.

Computes: out[b,s,h,d] = x[b,s,:] @ W_orig[:,h,d] + SCALE * (x @ A) @ B[:,h,d]

Strategy (data-parallel over tokens, LoRA as extra PSUM accumulation):
  - Shard x over tokens (B*S = 8192 -> 1024 per core); W/A/B replicated.
  - Main matmul: out[1024, 2048] = x_slice @ W, accumulated over 16 k-tiles
    into PSUM bank-pairs; 2 token tiles in flight across the 8 banks.
  - LoRA path on device: axT = (SCALE*A).T @ x [4, 1024] on the PE (hidden
    in the DMA-bound startup window), then each output chunk takes a 17th
    accumulating matmul  psum += axT_tile.T @ B_chunk  (start=False).
  - All matmul operands are bf16 (cast on host: halves DMA traffic, PE at
    1 row/cycle); PSUM accumulates fp32; output staged bf16 and upcast to
    fp32 on the host.
  - x ships pre-transposed as [128, k, t] k-slabs so the contraction dim
    lands on SBUF partitions and DMA descriptors stay >= 512B contiguous.
    x is split x01 (first 2 token tiles) / xrest so the first tiles and
    all of W can stream in ahead of the rest.
"""

import numpy as np

# Problem shapes (hardcoded per contract - kernel.py must be self-contained)
B, S, H = 4, 2048, 2048
NH, HD = 16, 128
N = NH * HD            # 2048 output features
RANK = 4
ALPHA = 4.0
SCALE = ALPHA / RANK   # 1.0
NCORES = 8
TOK = B * S            # 8192 tokens total
TPC = TOK // NCORES    # 1024 tokens per core

P = 128                # SBUF partitions
KT = H // P            # 16 contraction tiles
TT = TPC // P          # 8 token tiles per core
CH = 512               # psum bank width (fp32)
NCH = N // CH          # 4 chunks

_CACHE = {}


def _build_program():
    import concourse.mybir as mybir
    import concourse.tile as tile
    from concourse import bacc

    f32 = mybir.dt.float32
    bf16 = mybir.dt.bfloat16

    nc = bacc.Bacc(None, target_bir_lowering=False, debug=False)

    x01 = nc.dram_tensor("x01", [P, KT, 2 * P], bf16, kind="ExternalInput")
    xrest = nc.dram_tensor("xrest", [P, KT, 6 * P], bf16, kind="ExternalInput")
    w = nc.dram_tensor("w", [KT, P, N], bf16, kind="ExternalInput")
    ak = nc.dram_tensor("ak", [P, KT, RANK], bf16, kind="ExternalInput")
    bk = nc.dram_tensor("bk", [RANK, N], bf16, kind="ExternalInput")
    out = nc.dram_tensor("out", [TPC, N], bf16, kind="ExternalOutput")

    with tile.TileContext(nc) as tc:
        with (
            tc.tile_pool(name="wpool", bufs=1) as wpool,
            tc.tile_pool(name="xpool", bufs=1) as xpool,
            tc.tile_pool(name="cpool", bufs=1) as cpool,
            tc.tile_pool(name="opool", bufs=2) as opool,
            tc.tile_pool(name="psum", bufs=1, space="PSUM") as psum,
        ):
            # ---- input DMAs (sync queue, priority order) ----
            ak_sb = cpool.tile([P, KT, RANK], bf16, tag="ak", name="ak_sb")
            nc.sync.dma_start(ak_sb[:], ak[:])
            bk_sb = cpool.tile([RANK, N], bf16, tag="bk", name="bk_sb")
            nc.sync.dma_start(bk_sb[:], bk[:])

            x01_sb = xpool.tile([P, KT, 2 * P], bf16, tag="x01", name="x01_sb")
            nc.sync.dma_start(x01_sb[:], x01[:])

            w_sb = []
            for k in range(KT):
                wt = wpool.tile([P, N], bf16, tag=f"w{k}", name=f"w_{k}")
                nc.sync.dma_start(wt[:], w[k])
                w_sb.append(wt)

            xrest_sb = xpool.tile([P, KT, 6 * P], bf16, tag="xr",
                                  name="xrest_sb")
            nc.sync.dma_start(xrest_sb[:], xrest[:])

            # axT staging in SBUF: [4, TPC] bf16
            axt_sb = cpool.tile([RANK, TPC], bf16, tag="axt", name="axt_sb")

            # PSUM: 4 bank-pair tags (2 banks each = 8 banks).
            # q0/q1: even token tiles; q2/q3: odd tiles; q3 ring also hosts
            # the two axT stages (before t1c23 / before t3).
            def open_pair(tag, name):
                return psum.tile([P, 2 * CH], f32, tag=tag, name=name)

            def ax_stage(x_sb, width, dst_off, name):
                """axT[:, dst_off:dst_off+width] = (SCALE*A).T @ x_sb tokens."""
                qp = open_pair("q3", name)
                for off in range(0, width, CH):
                    wdt = min(CH, width - off)
                    for k in range(KT):
                        nc.tensor.matmul(
                            qp[0:RANK, off:off + wdt],
                            ak_sb[:, k, :],
                            x_sb[:, k, off:off + wdt],
                            start=(k == 0), stop=(k == KT - 1),
                        )
                nc.vector.tensor_copy(
                    axt_sb[:, dst_off:dst_off + width], qp[0:RANK, 0:width])

            q_t = {}

            def t_x(t):
                return (x01_sb, t) if t < 2 else (xrest_sb, t - 2)

            def main_mm01(t, k):
                qa = q_t[t][0]
                x_sb, toff = t_x(t)
                lhsT = x_sb[:, k, toff * P:(toff + 1) * P]
                st, sp = (k == 0), False
                nc.tensor.matmul(qa[:, 0:CH], lhsT, w_sb[k][:, 0:CH],
                                 start=st, stop=sp)
                nc.tensor.matmul(qa[:, CH:2 * CH], lhsT, w_sb[k][:, CH:2 * CH],
                                 start=st, stop=sp)

            def main_mm23(t, k):
                qb = q_t[t][1]
                x_sb, toff = t_x(t)
                lhsT = x_sb[:, k, toff * P:(toff + 1) * P]
                st, sp = (k == 0), False
                nc.tensor.matmul(qb[:, 0:CH], lhsT, w_sb[k][:, 2 * CH:3 * CH],
                                 start=st, stop=sp)
                nc.tensor.matmul(qb[:, CH:2 * CH], lhsT,
                                 w_sb[k][:, 3 * CH:4 * CH], start=st, stop=sp)

            def lora01(t):
                qa = q_t[t][0]
                a_sl = axt_sb[:, t * P:(t + 1) * P]
                nc.tensor.matmul(qa[:, 0:CH], a_sl, bk_sb[:, 0:CH],
                                 start=False, stop=True)
                nc.tensor.matmul(qa[:, CH:2 * CH], a_sl, bk_sb[:, CH:2 * CH],
                                 start=False, stop=True)

            def lora23(t):
                qb = q_t[t][1]
                a_sl = axt_sb[:, t * P:(t + 1) * P]
                nc.tensor.matmul(qb[:, 0:CH], a_sl, bk_sb[:, 2 * CH:3 * CH],
                                 start=False, stop=True)
                nc.tensor.matmul(qb[:, CH:2 * CH], a_sl,
                                 bk_sb[:, 3 * CH:4 * CH], start=False,
                                 stop=True)

            def main_close(t):
                qa, qb = q_t.pop(t)
                ot = opool.tile([P, N], bf16, tag="o", name=f"o_{t}")
                nc.vector.tensor_copy(ot[:, 0:2 * CH], qa[:])
                nc.scalar.copy(ot[:, 2 * CH:4 * CH], qb[:])
                nc.gpsimd.dma_start(out[t * P:(t + 1) * P, :], ot[:])

            # ---- startup: axT for tiles 0-1, then t0 + t1c01 interleaved --
            ax_stage(x01_sb, 2 * P, 0, "qax01")

            q_t[0] = (open_pair("q0", "qm01_0"), open_pair("q1", "qm23_0"))
            q_t[1] = (open_pair("q2", "qm01_1"), None)
            for k in range(KT):
                main_mm01(0, k)
                main_mm23(0, k)
                main_mm01(1, k)
            lora01(0)
            lora23(0)
            main_close(0)

            # t1 second half + close
            q_t[1] = (q_t[1][0], open_pair("q3", "qm23_1"))
            for k in range(KT):
                main_mm23(1, k)
            lora01(1)
            lora23(1)
            main_close(1)

            # axT for tiles 2-7 (two psum chunks in one q3 ring slot)
            ax_stage(xrest_sb, 6 * P, 2 * P, "qax27")

            # ---- remaining token tiles ----
            for t in range(2, TT):
                e = 2 * (t % 2)
                q_t[t] = (open_pair(f"q{e}", f"qm01_{t}"),
                          open_pair(f"q{e + 1}", f"qm23_{t}"))
                for k in range(KT):
                    main_mm01(t, k)
                    main_mm23(t, k)
                lora01(t)
                lora23(t)
                main_close(t)

    nc.compile()
    return nc


def _prep_inputs(x, W_orig, A_kernel, B_kernel):
    import ml_dtypes

    bf16 = ml_dtypes.bfloat16
    x = np.asarray(x, dtype=np.float32)
    W_orig = np.asarray(W_orig, dtype=np.float32)
    A_kernel = np.asarray(A_kernel, dtype=np.float32)
    B_kernel = np.asarray(B_kernel, dtype=np.float32)

    w3 = np.ascontiguousarray(
        W_orig.reshape(KT, P, N).astype(bf16))             # [KT, P, N]
    akm = np.ascontiguousarray(
        (A_kernel * np.float32(SCALE)).reshape(KT, P, RANK)
        .transpose(1, 0, 2).astype(bf16))                  # [P, KT, RANK]
    bkm = np.ascontiguousarray(
        B_kernel.reshape(RANK, N).astype(bf16))            # [RANK, N]

    x2d = x.reshape(TOK, H)
    in_maps = []
    for i in range(NCORES):
        xs = x2d[i * TPC:(i + 1) * TPC].T                  # [H, TPC]
        xs = xs.reshape(KT, P, TPC).transpose(1, 0, 2)     # [P, KT, TPC]
        xs = xs.astype(bf16)
        in_maps.append({
            "x01": np.ascontiguousarray(xs[:, :, 0:2 * P]),
            "xrest": np.ascontiguousarray(xs[:, :, 2 * P:]),
            "w": w3,
            "ak": akm,
            "bk": bkm,
        })
    return in_maps


def kernel(x, W_orig, A_kernel, B_kernel):
    from concourse.bass_utils import run_bass_kernel_spmd

    if "nc" not in _CACHE:
        _CACHE["nc"] = _build_program()
    nc = _CACHE["nc"]

    in_maps = _prep_inputs(x, W_orig, A_kernel, B_kernel)
    res = run_bass_kernel_spmd(nc, in_maps, list(range(NCORES)))
    parts = [np.asarray(res.results[i]["out"]) for i in range(NCORES)]
    full = np.concatenate(parts, axis=0).astype(np.float32)   # [TOK, N]
    return full.reshape(B, S, NH, HD)


# revision 8
# speedup vs baseline: 1.2734x; 1.0267x over previous
"""LoRA layer kernel for Trainium2, SPMD across 8 NeuronCores.

Computes: out[b,s,h,d] = x[b,s,:] @ W_orig[:,h,d] + SCALE * (x @ A) @ B[:,h,d]

Strategy (data-parallel over tokens, LoRA as extra PSUM accumulation):
  - Shard x over tokens (B*S = 8192 -> 1024 per core); W/A/B replicated.
  - Main matmul: out[1024, 2048] = x_slice @ W, accumulated over 16 k-tiles
    into PSUM bank-pairs; 2 token tiles in flight across the 8 banks.
  - LoRA path on device: axT = (SCALE*A).T @ x [4, 1024] on the PE (hidden
    in the DMA-bound startup window), then each output chunk takes a 17th
    accumulating matmul  psum += axT_tile.T @ B_chunk  (start=False).
  - All matmul operands are bf16 (cast on host: halves DMA traffic, PE at
    1 row/cycle); PSUM accumulates fp32; output staged bf16 and upcast to
    fp32 on the host.
  - x ships pre-transposed as [128, k, t] k-slabs so the contraction dim
    lands on SBUF partitions and DMA descriptors stay >= 512B contiguous.
    x is split x01 (first 2 token tiles) / xrest so the first tiles and
    all of W can stream in ahead of the rest.
"""

import numpy as np

# Problem shapes (hardcoded per contract - kernel.py must be self-contained)
B, S, H = 4, 2048, 2048
NH, HD = 16, 128
N = NH * HD            # 2048 output features
RANK = 4
ALPHA = 4.0
SCALE = ALPHA / RANK   # 1.0
NCORES = 8
TOK = B * S            # 8192 tokens total
TPC = TOK // NCORES    # 1024 tokens per core

P = 128                # SBUF partitions
KT = H // P            # 16 contraction tiles
TT = TPC // P          # 8 token tiles per core
CH = 512               # psum bank width (fp32)
NCH = N // CH          # 4 chunks

_CACHE = {}


def _build_program():
    import concourse.mybir as mybir
    import concourse.tile as tile
    from concourse import bacc

    f32 = mybir.dt.float32
    bf16 = mybir.dt.bfloat16

    nc = bacc.Bacc(None, target_bir_lowering=False, debug=False)

    x01 = nc.dram_tensor("x01", [P, KT, 2 * P], bf16, kind="ExternalInput")
    xr23 = nc.dram_tensor("xr23", [P, KT, 2 * P], bf16, kind="ExternalInput")
    xr47 = nc.dram_tensor("xr47", [P, KT, 4 * P], bf16, kind="ExternalInput")
    w = nc.dram_tensor("w", [KT, P, N], bf16, kind="ExternalInput")
    ak = nc.dram_tensor("ak", [P, KT * RANK], bf16, kind="ExternalInput")
    bk = nc.dram_tensor("bk", [RANK, N], bf16, kind="ExternalInput")
    out = nc.dram_tensor("out", [TPC, N], bf16, kind="ExternalOutput")

    with tile.TileContext(nc) as tc:
        with (
            tc.tile_pool(name="wpool", bufs=1) as wpool,
            tc.tile_pool(name="xpool", bufs=1) as xpool,
            tc.tile_pool(name="cpool", bufs=1) as cpool,
            tc.tile_pool(name="opool", bufs=2) as opool,
            tc.tile_pool(name="psum", bufs=1, space="PSUM") as psum,
        ):
            # ---- input DMAs (sync queue, priority order) ----
            x01_sb = xpool.tile([P, KT, 2 * P], bf16, tag="x01", name="x01_sb")
            nc.sync.dma_start(x01_sb[:], x01[:])
            ak_sb = cpool.tile([P, KT * RANK], bf16, tag="ak", name="ak_sb")
            nc.sync.dma_start(ak_sb[:], ak[:])
            bk_sb = cpool.tile([RANK, N], bf16, tag="bk", name="bk_sb")
            nc.sync.dma_start(bk_sb[:], bk[:])

            w_sb = []
            for k in range(KT):
                wt = wpool.tile([P, N], bf16, tag=f"w{k}", name=f"w_{k}")
                nc.sync.dma_start(wt[:], w[k])
                w_sb.append(wt)

            xr23_sb = xpool.tile([P, KT, 2 * P], bf16, tag="xr23",
                                 name="xr23_sb")
            nc.sync.dma_start(xr23_sb[:], xr23[:])
            xr47_sb = xpool.tile([P, KT, 4 * P], bf16, tag="xr47",
                                 name="xr47_sb")
            nc.sync.dma_start(xr47_sb[:], xr47[:])

            # axT staging in SBUF: [4, TPC] bf16
            axt_sb = cpool.tile([RANK, TPC], bf16, tag="axt", name="axt_sb")

            # PSUM: 4 bank-pair tags (2 banks each = 8 banks).
            # q0/q1: even token tiles; q2/q3: odd tiles; q3 ring also hosts
            # the two axT stages (before t1c23 / before t3).
            def open_pair(tag, name):
                return psum.tile([P, 2 * CH], f32, tag=tag, name=name)

            def ax_stage(x_sb, width, dst_off, name):
                """axT[:, dst_off:dst_off+width] = (SCALE*A).T @ x_sb tokens."""
                qp = open_pair("q3", name)
                for off in range(0, width, CH):
                    wdt = min(CH, width - off)
                    for k in range(KT):
                        nc.tensor.matmul(
                            qp[0:RANK, off:off + wdt],
                            ak_sb[:, k * RANK:(k + 1) * RANK],
                            x_sb[:, k, off:off + wdt],
                            start=(k == 0), stop=(k == KT - 1),
                        )
                nc.vector.tensor_copy(
                    axt_sb[:, dst_off:dst_off + width], qp[0:RANK, 0:width])

            q_t = {}

            def t_x(t):
                if t < 2:
                    return (x01_sb, t)
                if t < 4:
                    return (xr23_sb, t - 2)
                return (xr47_sb, t - 4)

            def main_mm01(t, k):
                qa = q_t[t][0]
                x_sb, toff = t_x(t)
                lhsT = x_sb[:, k, toff * P:(toff + 1) * P]
                st, sp = (k == 0), False
                nc.tensor.matmul(qa[:, 0:CH], lhsT, w_sb[k][:, 0:CH],
                                 start=st, stop=sp)
                nc.tensor.matmul(qa[:, CH:2 * CH], lhsT, w_sb[k][:, CH:2 * CH],
                                 start=st, stop=sp)

            def main_mm23(t, k):
                qb = q_t[t][1]
                x_sb, toff = t_x(t)
                lhsT = x_sb[:, k, toff * P:(toff + 1) * P]
                st, sp = (k == 0), False
                nc.tensor.matmul(qb[:, 0:CH], lhsT, w_sb[k][:, 2 * CH:3 * CH],
                                 start=st, stop=sp)
                nc.tensor.matmul(qb[:, CH:2 * CH], lhsT,
                                 w_sb[k][:, 3 * CH:4 * CH], start=st, stop=sp)

            def lora01(t):
                qa = q_t[t][0]
                a_sl = axt_sb[:, t * P:(t + 1) * P]
                nc.tensor.matmul(qa[:, 0:CH], a_sl, bk_sb[:, 0:CH],
                                 start=False, stop=True)
                nc.tensor.matmul(qa[:, CH:2 * CH], a_sl, bk_sb[:, CH:2 * CH],
                                 start=False, stop=True)

            def lora23(t):
                qb = q_t[t][1]
                a_sl = axt_sb[:, t * P:(t + 1) * P]
                nc.tensor.matmul(qb[:, 0:CH], a_sl, bk_sb[:, 2 * CH:3 * CH],
                                 start=False, stop=True)
                nc.tensor.matmul(qb[:, CH:2 * CH], a_sl,
                                 bk_sb[:, 3 * CH:4 * CH], start=False,
                                 stop=True)

            def main_close(t):
                qa, qb = q_t.pop(t)
                ot = opool.tile([P, N], bf16, tag="o", name=f"o_{t}")
                nc.vector.tensor_copy(ot[:, 0:2 * CH], qa[:])
                nc.scalar.copy(ot[:, 2 * CH:4 * CH], qb[:])
                nc.gpsimd.dma_start(out[t * P:(t + 1) * P, :], ot[:])

            # ---- startup: axT for tiles 0-1, then t0 + t1c01 interleaved --
            ax_stage(x01_sb, 2 * P, 0, "qax01")

            q_t[0] = (open_pair("q0", "qm01_0"), open_pair("q1", "qm23_0"))
            q_t[1] = (open_pair("q2", "qm01_1"), None)
            for k in range(KT):
                main_mm01(0, k)
                main_mm23(0, k)
                main_mm01(1, k)
            lora01(0)
            lora23(0)
            main_close(0)

            # t1 second half + close
            q_t[1] = (q_t[1][0], open_pair("q3", "qm23_1"))
            for k in range(KT):
                main_mm23(1, k)
            lora01(1)
            lora23(1)
            main_close(1)

            # axT for tiles 2-3 / 4-7, interleaved with the tiles they feed
            ax_stage(xr23_sb, 2 * P, 2 * P, "qax23")

            def run_tile(t):
                e = 2 * (t % 2)
                q_t[t] = (open_pair(f"q{e}", f"qm01_{t}"),
                          open_pair(f"q{e + 1}", f"qm23_{t}"))
                for k in range(KT):
                    main_mm01(t, k)
                    main_mm23(t, k)
                lora01(t)
                lora23(t)
                main_close(t)

            run_tile(2)
            ax_stage(xr47_sb, 4 * P, 4 * P, "qax47")
            for t in range(3, TT - 1):
                run_tile(t)

            # last tile: close each bank-pair as soon as its half is done,
            # so the second half's matmuls overlap the first half's copy+DMA
            t = TT - 1
            e = 2 * (t % 2)
            q_t[t] = (open_pair(f"q{e}", f"qm01_{t}"), None)
            for k in range(KT):
                main_mm01(t, k)
            lora01(t)
            qa = q_t[t][0]
            ot = opool.tile([P, N], bf16, tag="o", name=f"o_{t}")
            nc.vector.tensor_copy(ot[:, 0:2 * CH], qa[:])
            nc.gpsimd.dma_start(out[t * P:(t + 1) * P, 0:2 * CH],
                                ot[:, 0:2 * CH])
            q_t[t] = (qa, open_pair(f"q{e + 1}", f"qm23_{t}"))
            for k in range(KT):
                main_mm23(t, k)
            lora23(t)
            qb = q_t.pop(t)[1]
            nc.scalar.copy(ot[:, 2 * CH:4 * CH], qb[:])
            nc.gpsimd.dma_start(out[t * P:(t + 1) * P, 2 * CH:4 * CH],
                                ot[:, 2 * CH:4 * CH])

    nc.compile()
    return nc


def _prep_inputs(x, W_orig, A_kernel, B_kernel):
    import ml_dtypes

    bf16 = ml_dtypes.bfloat16
    x = np.asarray(x, dtype=np.float32)
    W_orig = np.asarray(W_orig, dtype=np.float32)
    A_kernel = np.asarray(A_kernel, dtype=np.float32)
    B_kernel = np.asarray(B_kernel, dtype=np.float32)

    w3 = np.ascontiguousarray(
        W_orig.reshape(KT, P, N).astype(bf16))             # [KT, P, N]
    akm = np.ascontiguousarray(
        (A_kernel * np.float32(SCALE)).reshape(KT, P, RANK)
        .transpose(1, 0, 2).reshape(P, KT * RANK)
        .astype(bf16))                                     # [P, KT*RANK]
    bkm = np.ascontiguousarray(
        B_kernel.reshape(RANK, N).astype(bf16))            # [RANK, N]

    x2d = x.reshape(TOK, H)
    in_maps = []
    for i in range(NCORES):
        xs = x2d[i * TPC:(i + 1) * TPC].T                  # [H, TPC]
        xs = xs.reshape(KT, P, TPC).transpose(1, 0, 2)     # [P, KT, TPC]
        xs = xs.astype(bf16)
        in_maps.append({
            "x01": np.ascontiguousarray(xs[:, :, 0:2 * P]),
            "xr23": np.ascontiguousarray(xs[:, :, 2 * P:4 * P]),
            "xr47": np.ascontiguousarray(xs[:, :, 4 * P:]),
            "w": w3,
            "ak": akm,
            "bk": bkm,
        })
    return in_maps


def kernel(x, W_orig, A_kernel, B_kernel):
    from concourse.bass_utils import run_bass_kernel_spmd

    if "nc" not in _CACHE:
        _CACHE["nc"] = _build_program()
    nc = _CACHE["nc"]

    in_maps = _prep_inputs(x, W_orig, A_kernel, B_kernel)
    res = run_bass_kernel_spmd(nc, in_maps, list(range(NCORES)))
    parts = [np.asarray(res.results[i]["out"]) for i in range(NCORES)]
    full = np.concatenate(parts, axis=0).astype(np.float32)   # [TOK, N]
    return full.reshape(B, S, NH, HD)


# revision 14
# speedup vs baseline: 1.2760x; 1.0020x over previous
"""LoRA layer kernel for Trainium2, SPMD across 8 NeuronCores.

Computes: out[b,s,h,d] = x[b,s,:] @ W_orig[:,h,d] + SCALE * (x @ A) @ B[:,h,d]

Strategy (data-parallel over tokens, LoRA as extra PSUM accumulation):
  - Shard x over tokens (B*S = 8192 -> 1024 per core); W/A/B replicated.
  - Main matmul: out[1024, 2048] = x_slice @ W, accumulated over 16 k-tiles
    into PSUM bank-pairs; 2 token tiles in flight across the 8 banks.
  - LoRA path on device: axT = (SCALE*A).T @ x [4, 1024] on the PE (hidden
    in the DMA-bound startup window), then each output chunk takes a 17th
    accumulating matmul  psum += axT_tile.T @ B_chunk  (start=False).
  - All matmul operands are bf16 (cast on host: halves DMA traffic, PE at
    1 row/cycle); PSUM accumulates fp32; output staged bf16 and upcast to
    fp32 on the host.
  - x ships pre-transposed as [128, k, t] k-slabs so the contraction dim
    lands on SBUF partitions and DMA descriptors stay >= 512B contiguous.
    x is split x01 (first 2 token tiles) / xrest so the first tiles and
    all of W can stream in ahead of the rest.
"""

import numpy as np

# Problem shapes (hardcoded per contract - kernel.py must be self-contained)
B, S, H = 4, 2048, 2048
NH, HD = 16, 128
N = NH * HD            # 2048 output features
RANK = 4
ALPHA = 4.0
SCALE = ALPHA / RANK   # 1.0
NCORES = 8
TOK = B * S            # 8192 tokens total
TPC = TOK // NCORES    # 1024 tokens per core

P = 128                # SBUF partitions
KT = H // P            # 16 contraction tiles
TT = TPC // P          # 8 token tiles per core
CH = 512               # psum bank width (fp32)
NCH = N // CH          # 4 chunks

_CACHE = {}


def _build_program():
    import concourse.mybir as mybir
    import concourse.tile as tile
    from concourse import bacc

    f32 = mybir.dt.float32
    bf16 = mybir.dt.bfloat16

    nc = bacc.Bacc(None, target_bir_lowering=False, debug=False)

    x0 = nc.dram_tensor("x0", [P, KT, P], bf16, kind="ExternalInput")
    x1 = nc.dram_tensor("x1", [P, KT, P], bf16, kind="ExternalInput")
    xr23 = nc.dram_tensor("xr23", [P, KT, 2 * P], bf16, kind="ExternalInput")
    xr47 = nc.dram_tensor("xr47", [P, KT, 4 * P], bf16, kind="ExternalInput")
    w = nc.dram_tensor("w", [KT, P, N], bf16, kind="ExternalInput")
    ak = nc.dram_tensor("ak", [P, KT * RANK], bf16, kind="ExternalInput")
    bk = nc.dram_tensor("bk", [RANK, N], bf16, kind="ExternalInput")
    out = nc.dram_tensor("out", [TPC, N], bf16, kind="ExternalOutput")

    with tile.TileContext(nc) as tc:
        with (
            tc.tile_pool(name="wpool", bufs=1) as wpool,
            tc.tile_pool(name="xpool", bufs=1) as xpool,
            tc.tile_pool(name="cpool", bufs=1) as cpool,
            tc.tile_pool(name="opool", bufs=2) as opool,
            tc.tile_pool(name="psum", bufs=1, space="PSUM") as psum,
        ):
            # ---- input DMAs (sync queue, priority order) ----
            x0_sb = xpool.tile([P, KT, P], bf16, tag="x0", name="x0_sb")
            nc.sync.dma_start(x0_sb[:], x0[:])
            ak_sb = cpool.tile([P, KT * RANK], bf16, tag="ak", name="ak_sb")
            nc.sync.dma_start(ak_sb[:], ak[:])
            bk_sb = cpool.tile([RANK, N], bf16, tag="bk", name="bk_sb")
            nc.sync.dma_start(bk_sb[:], bk[:])

            w_sb = []

            def load_w(k):
                wt = wpool.tile([P, N], bf16, tag=f"w{k}", name=f"w_{k}")
                nc.sync.dma_start(wt[:], w[k])
                w_sb.append(wt)

            load_w(0)
            x1_sb = xpool.tile([P, KT, P], bf16, tag="x1", name="x1_sb")
            nc.sync.dma_start(x1_sb[:], x1[:])
            for k in range(1, KT):
                load_w(k)

            xr23_sb = xpool.tile([P, KT, 2 * P], bf16, tag="xr23",
                                 name="xr23_sb")
            nc.sync.dma_start(xr23_sb[:], xr23[:])
            xr47_sb = xpool.tile([P, KT, 4 * P], bf16, tag="xr47",
                                 name="xr47_sb")
            nc.sync.dma_start(xr47_sb[:], xr47[:])

            # axT staging in SBUF: [4, TPC] bf16
            axt_sb = cpool.tile([RANK, TPC], bf16, tag="axt", name="axt_sb")

            # PSUM: 4 bank-pair tags (2 banks each = 8 banks).
            # q0/q1: even token tiles; q2/q3: odd tiles; q3 ring also hosts
            # the two axT stages (before t1c23 / before t3).
            def open_pair(tag, name):
                return psum.tile([P, 2 * CH], f32, tag=tag, name=name)

            def ax_stage(x_sb, width, dst_off, name):
                """axT[:, dst_off:dst_off+width] = (SCALE*A).T @ x_sb tokens."""
                qp = open_pair("q3", name)
                for off in range(0, width, CH):
                    wdt = min(CH, width - off)
                    for k in range(KT):
                        nc.tensor.matmul(
                            qp[0:RANK, off:off + wdt],
                            ak_sb[:, k * RANK:(k + 1) * RANK],
                            x_sb[:, k, off:off + wdt],
                            start=(k == 0), stop=(k == KT - 1),
                        )
                nc.vector.tensor_copy(
                    axt_sb[:, dst_off:dst_off + width], qp[0:RANK, 0:width])

            q_t = {}

            def t_x(t):
                if t == 0:
                    return (x0_sb, 0)
                if t == 1:
                    return (x1_sb, 0)
                if t < 4:
                    return (xr23_sb, t - 2)
                return (xr47_sb, t - 4)

            def main_mm01(t, k):
                qa = q_t[t][0]
                x_sb, toff = t_x(t)
                lhsT = x_sb[:, k, toff * P:(toff + 1) * P]
                st, sp = (k == 0), False
                nc.tensor.matmul(qa[:, 0:CH], lhsT, w_sb[k][:, 0:CH],
                                 start=st, stop=sp)
                nc.tensor.matmul(qa[:, CH:2 * CH], lhsT, w_sb[k][:, CH:2 * CH],
                                 start=st, stop=sp)

            def main_mm23(t, k):
                qb = q_t[t][1]
                x_sb, toff = t_x(t)
                lhsT = x_sb[:, k, toff * P:(toff + 1) * P]
                st, sp = (k == 0), False
                nc.tensor.matmul(qb[:, 0:CH], lhsT, w_sb[k][:, 2 * CH:3 * CH],
                                 start=st, stop=sp)
                nc.tensor.matmul(qb[:, CH:2 * CH], lhsT,
                                 w_sb[k][:, 3 * CH:4 * CH], start=st, stop=sp)

            def lora01(t):
                qa = q_t[t][0]
                a_sl = axt_sb[:, t * P:(t + 1) * P]
                nc.tensor.matmul(qa[:, 0:CH], a_sl, bk_sb[:, 0:CH],
                                 start=False, stop=True)
                nc.tensor.matmul(qa[:, CH:2 * CH], a_sl, bk_sb[:, CH:2 * CH],
                                 start=False, stop=True)

            def lora23(t):
                qb = q_t[t][1]
                a_sl = axt_sb[:, t * P:(t + 1) * P]
                nc.tensor.matmul(qb[:, 0:CH], a_sl, bk_sb[:, 2 * CH:3 * CH],
                                 start=False, stop=True)
                nc.tensor.matmul(qb[:, CH:2 * CH], a_sl,
                                 bk_sb[:, 3 * CH:4 * CH], start=False,
                                 stop=True)

            def main_close(t):
                qa, qb = q_t.pop(t)
                ot = opool.tile([P, N], bf16, tag="o", name=f"o_{t}")
                nc.vector.tensor_copy(ot[:, 0:2 * CH], qa[:])
                nc.scalar.copy(ot[:, 2 * CH:4 * CH], qb[:])
                nc.gpsimd.dma_start(out[t * P:(t + 1) * P, :], ot[:])

            # ---- startup: axT(t0), first t0 matmuls, axT(t1), then the
            # interleaved t0 + t1c01 k-loop (t1 lags 2 so x1 can land) ----
            ax_stage(x0_sb, P, 0, "qax0")

            q_t[0] = (open_pair("q0", "qm01_0"), open_pair("q1", "qm23_0"))
            q_t[1] = (open_pair("q2", "qm01_1"), None)
            main_mm01(0, 0)
            main_mm23(0, 0)
            main_mm01(0, 1)
            main_mm23(0, 1)
            ax_stage(x1_sb, P, P, "qax1")
            for k in range(2, KT):
                main_mm01(0, k)
                main_mm23(0, k)
                main_mm01(1, k - 2)
            main_mm01(1, KT - 2)
            main_mm01(1, KT - 1)
            lora01(0)
            lora23(0)
            main_close(0)

            # t1 second half + close
            q_t[1] = (q_t[1][0], open_pair("q3", "qm23_1"))
            for k in range(KT):
                main_mm23(1, k)
            lora01(1)
            lora23(1)
            main_close(1)

            # axT for tiles 2-3 / 4-7, interleaved with the tiles they feed
            ax_stage(xr23_sb, 2 * P, 2 * P, "qax23")

            def run_tile(t):
                e = 2 * (t % 2)
                q_t[t] = (open_pair(f"q{e}", f"qm01_{t}"),
                          open_pair(f"q{e + 1}", f"qm23_{t}"))
                for k in range(KT):
                    main_mm01(t, k)
                    main_mm23(t, k)
                lora01(t)
                lora23(t)
                main_close(t)

            run_tile(2)
            ax_stage(xr47_sb, 4 * P, 4 * P, "qax47")
            for t in range(3, TT - 1):
                run_tile(t)

            # last tile: close each bank-pair as soon as its half is done,
            # so the second half's matmuls overlap the first half's copy+DMA
            t = TT - 1
            e = 2 * (t % 2)
            q_t[t] = (open_pair(f"q{e}", f"qm01_{t}"), None)
            for k in range(KT):
                main_mm01(t, k)
            lora01(t)
            qa = q_t[t][0]
            ot = opool.tile([P, N], bf16, tag="o", name=f"o_{t}")
            nc.vector.tensor_copy(ot[:, 0:2 * CH], qa[:])
            nc.gpsimd.dma_start(out[t * P:(t + 1) * P, 0:2 * CH],
                                ot[:, 0:2 * CH])
            # c2 then c3 separately: each chunk's copy+DMA overlaps the next
            # chunk's matmuls (subtile deps let the c2 copy start early)
            qb = open_pair(f"q{e + 1}", f"qm23_{t}")
            q_t[t] = (qa, qb)
            x_sb, toff = t_x(t)
            a_sl = axt_sb[:, t * P:(t + 1) * P]
            for c in (2, 3):
                lo = (c - 2) * CH
                for k in range(KT):
                    lhsT = x_sb[:, k, toff * P:(toff + 1) * P]
                    nc.tensor.matmul(qb[:, lo:lo + CH], lhsT,
                                     w_sb[k][:, c * CH:(c + 1) * CH],
                                     start=(k == 0), stop=False)
                nc.tensor.matmul(qb[:, lo:lo + CH], a_sl,
                                 bk_sb[:, c * CH:(c + 1) * CH],
                                 start=False, stop=True)
                nc.scalar.copy(ot[:, c * CH:(c + 1) * CH], qb[:, lo:lo + CH])
                nc.gpsimd.dma_start(out[t * P:(t + 1) * P, c * CH:(c + 1) * CH],
                                    ot[:, c * CH:(c + 1) * CH])
            q_t.pop(t)

    nc.compile()
    return nc


def _prep_inputs(x, W_orig, A_kernel, B_kernel):
    import ml_dtypes

    bf16 = ml_dtypes.bfloat16
    x = np.asarray(x, dtype=np.float32)
    W_orig = np.asarray(W_orig, dtype=np.float32)
    A_kernel = np.asarray(A_kernel, dtype=np.float32)
    B_kernel = np.asarray(B_kernel, dtype=np.float32)

    w3 = np.ascontiguousarray(
        W_orig.reshape(KT, P, N).astype(bf16))             # [KT, P, N]
    akm = np.ascontiguousarray(
        (A_kernel * np.float32(SCALE)).reshape(KT, P, RANK)
        .transpose(1, 0, 2).reshape(P, KT * RANK)
        .astype(bf16))                                     # [P, KT*RANK]
    bkm = np.ascontiguousarray(
        B_kernel.reshape(RANK, N).astype(bf16))            # [RANK, N]

    x2d = x.reshape(TOK, H)
    in_maps = []
    for i in range(NCORES):
        xs = x2d[i * TPC:(i + 1) * TPC].T                  # [H, TPC]
        xs = xs.reshape(KT, P, TPC).transpose(1, 0, 2)     # [P, KT, TPC]
        xs = xs.astype(bf16)
        in_maps.append({
            "x0": np.ascontiguousarray(xs[:, :, 0:P]),
            "x1": np.ascontiguousarray(xs[:, :, P:2 * P]),
            "xr23": np.ascontiguousarray(xs[:, :, 2 * P:4 * P]),
            "xr47": np.ascontiguousarray(xs[:, :, 4 * P:]),
            "w": w3,
            "ak": akm,
            "bk": bkm,
        })
    return in_maps


def kernel(x, W_orig, A_kernel, B_kernel):
    from concourse.bass_utils import run_bass_kernel_spmd

    if "nc" not in _CACHE:
        _CACHE["nc"] = _build_program()
    nc = _CACHE["nc"]

    in_maps = _prep_inputs(x, W_orig, A_kernel, B_kernel)
    res = run_bass_kernel_spmd(nc, in_maps, list(range(NCORES)))
    parts = [np.asarray(res.results[i]["out"]) for i in range(NCORES)]
    full = np.concatenate(parts, axis=0).astype(np.float32)   # [TOK, N]
    return full.reshape(B, S, NH, HD)


# revision 15
# speedup vs baseline: 1.2855x; 1.0075x over previous
"""LoRA layer kernel for Trainium2, SPMD across 8 NeuronCores.

Computes: out[b,s,h,d] = x[b,s,:] @ W_orig[:,h,d] + SCALE * (x @ A) @ B[:,h,d]

Strategy (data-parallel over tokens, LoRA as extra PSUM accumulation):
  - Shard x over tokens (B*S = 8192 -> 1024 per core); W/A/B replicated.
  - Main matmul: out[1024, 2048] = x_slice @ W, accumulated over 16 k-tiles
    into PSUM bank-pairs; 2 token tiles in flight across the 8 banks.
  - LoRA path on device: axT = (SCALE*A).T @ x [4, 1024] on the PE (hidden
    in the DMA-bound startup window), then each output chunk takes a 17th
    accumulating matmul  psum += axT_tile.T @ B_chunk  (start=False).
  - All matmul operands are bf16 (cast on host: halves DMA traffic, PE at
    1 row/cycle); PSUM accumulates fp32; output staged bf16 and upcast to
    fp32 on the host.
  - x ships pre-transposed as [128, k, t] k-slabs so the contraction dim
    lands on SBUF partitions and DMA descriptors stay >= 512B contiguous.
    x is split x01 (first 2 token tiles) / xrest so the first tiles and
    all of W can stream in ahead of the rest.
"""

import numpy as np

# Problem shapes (hardcoded per contract - kernel.py must be self-contained)
B, S, H = 4, 2048, 2048
NH, HD = 16, 128
N = NH * HD            # 2048 output features
RANK = 4
ALPHA = 4.0
SCALE = ALPHA / RANK   # 1.0
NCORES = 8
TOK = B * S            # 8192 tokens total
TPC = TOK // NCORES    # 1024 tokens per core

P = 128                # SBUF partitions
KT = H // P            # 16 contraction tiles
TT = TPC // P          # 8 token tiles per core
CH = 512               # psum bank width (fp32)
NCH = N // CH          # 4 chunks

_CACHE = {}


def _build_program():
    import concourse.mybir as mybir
    import concourse.tile as tile
    from concourse import bacc

    f32 = mybir.dt.float32
    bf16 = mybir.dt.bfloat16

    nc = bacc.Bacc(None, target_bir_lowering=False, debug=False)

    x0 = nc.dram_tensor("x0", [P, KT, P], bf16, kind="ExternalInput")
    x1 = nc.dram_tensor("x1", [P, KT, P], bf16, kind="ExternalInput")
    xr23 = nc.dram_tensor("xr23", [P, KT, 2 * P], bf16, kind="ExternalInput")
    xr47 = nc.dram_tensor("xr47", [P, KT, 4 * P], bf16, kind="ExternalInput")
    w = nc.dram_tensor("w", [KT, P, N], bf16, kind="ExternalInput")
    ak = nc.dram_tensor("ak", [P, KT * RANK], bf16, kind="ExternalInput")
    bk = nc.dram_tensor("bk", [RANK, N], bf16, kind="ExternalInput")
    out = nc.dram_tensor("out", [TPC, N], bf16, kind="ExternalOutput")

    with tile.TileContext(nc) as tc:
        with (
            tc.tile_pool(name="wpool", bufs=1) as wpool,
            tc.tile_pool(name="xpool", bufs=1) as xpool,
            tc.tile_pool(name="cpool", bufs=1) as cpool,
            tc.tile_pool(name="opool", bufs=2) as opool,
            tc.tile_pool(name="psum", bufs=1, space="PSUM") as psum,
        ):
            # ---- input DMAs (sync queue, priority order) ----
            x0_sb = xpool.tile([P, KT, P], bf16, tag="x0", name="x0_sb")
            nc.sync.dma_start(x0_sb[:], x0[:])
            ak_sb = cpool.tile([P, KT * RANK], bf16, tag="ak", name="ak_sb")
            nc.sync.dma_start(ak_sb[:], ak[:])
            bk_sb = cpool.tile([RANK, N], bf16, tag="bk", name="bk_sb")
            nc.sync.dma_start(bk_sb[:], bk[:])

            w_sb = []

            def load_w(k):
                wt = wpool.tile([P, N], bf16, tag=f"w{k}", name=f"w_{k}")
                nc.sync.dma_start(wt[:], w[k])
                w_sb.append(wt)

            load_w(0)
            x1_sb = xpool.tile([P, KT, P], bf16, tag="x1", name="x1_sb")
            nc.sync.dma_start(x1_sb[:], x1[:])
            for k in range(1, KT):
                load_w(k)

            xr23_sb = xpool.tile([P, KT, 2 * P], bf16, tag="xr23",
                                 name="xr23_sb")
            nc.sync.dma_start(xr23_sb[:], xr23[:])
            xr47_sb = xpool.tile([P, KT, 4 * P], bf16, tag="xr47",
                                 name="xr47_sb")
            nc.sync.dma_start(xr47_sb[:], xr47[:])

            # axT staging in SBUF: [4, TPC] bf16
            axt_sb = cpool.tile([RANK, TPC], bf16, tag="axt", name="axt_sb")

            # PSUM: 4 bank-pair tags (2 banks each = 8 banks).
            # q0/q1: even token tiles; q2/q3: odd tiles; q3 ring also hosts
            # the two axT stages (before t1c23 / before t3).
            def open_pair(tag, name):
                return psum.tile([P, 2 * CH], f32, tag=tag, name=name)

            def ax_stage(x_sb, width, dst_off, name):
                """axT[:, dst_off:dst_off+width] = (SCALE*A).T @ x_sb tokens."""
                qp = open_pair("q3", name)
                for off in range(0, width, CH):
                    wdt = min(CH, width - off)
                    for k in range(KT):
                        nc.tensor.matmul(
                            qp[0:RANK, off:off + wdt],
                            ak_sb[:, k * RANK:(k + 1) * RANK],
                            x_sb[:, k, off:off + wdt],
                            start=(k == 0), stop=(k == KT - 1),
                        )
                nc.vector.tensor_copy(
                    axt_sb[:, dst_off:dst_off + width], qp[0:RANK, 0:width])

            q_t = {}

            def t_x(t):
                if t == 0:
                    return (x0_sb, 0)
                if t == 1:
                    return (x1_sb, 0)
                if t < 4:
                    return (xr23_sb, t - 2)
                return (xr47_sb, t - 4)

            def main_mm01(t, k):
                qa = q_t[t][0]
                x_sb, toff = t_x(t)
                lhsT = x_sb[:, k, toff * P:(toff + 1) * P]
                st, sp = (k == 0), False
                nc.tensor.matmul(qa[:, 0:CH], lhsT, w_sb[k][:, 0:CH],
                                 start=st, stop=sp)
                nc.tensor.matmul(qa[:, CH:2 * CH], lhsT, w_sb[k][:, CH:2 * CH],
                                 start=st, stop=sp)

            def main_mm23(t, k):
                qb = q_t[t][1]
                x_sb, toff = t_x(t)
                lhsT = x_sb[:, k, toff * P:(toff + 1) * P]
                st, sp = (k == 0), False
                nc.tensor.matmul(qb[:, 0:CH], lhsT, w_sb[k][:, 2 * CH:3 * CH],
                                 start=st, stop=sp)
                nc.tensor.matmul(qb[:, CH:2 * CH], lhsT,
                                 w_sb[k][:, 3 * CH:4 * CH], start=st, stop=sp)

            def lora01(t):
                qa = q_t[t][0]
                a_sl = axt_sb[:, t * P:(t + 1) * P]
                nc.tensor.matmul(qa[:, 0:CH], a_sl, bk_sb[:, 0:CH],
                                 start=False, stop=True)
                nc.tensor.matmul(qa[:, CH:2 * CH], a_sl, bk_sb[:, CH:2 * CH],
                                 start=False, stop=True)

            def lora23(t):
                qb = q_t[t][1]
                a_sl = axt_sb[:, t * P:(t + 1) * P]
                nc.tensor.matmul(qb[:, 0:CH], a_sl, bk_sb[:, 2 * CH:3 * CH],
                                 start=False, stop=True)
                nc.tensor.matmul(qb[:, CH:2 * CH], a_sl,
                                 bk_sb[:, 3 * CH:4 * CH], start=False,
                                 stop=True)

            def main_close(t):
                qa, qb = q_t.pop(t)
                ot = opool.tile([P, N], bf16, tag="o", name=f"o_{t}")
                nc.vector.tensor_copy(ot[:, 0:2 * CH], qa[:])
                nc.scalar.copy(ot[:, 2 * CH:4 * CH], qb[:])
                nc.gpsimd.dma_start(out[t * P:(t + 1) * P, :], ot[:])

            # ---- startup: axT(t0), first t0 matmuls, axT(t1), then the
            # interleaved t0 + t1c01 k-loop (t1 lags 2 so x1 can land) ----
            ax_stage(x0_sb, P, 0, "qax0")

            q_t[0] = (open_pair("q0", "qm01_0"), open_pair("q1", "qm23_0"))
            q_t[1] = (open_pair("q2", "qm01_1"), None)
            main_mm01(0, 0)
            main_mm23(0, 0)
            main_mm01(0, 1)
            main_mm23(0, 1)
            ax_stage(x1_sb, P, P, "qax1")
            for k in range(2, KT):
                main_mm01(0, k)
                main_mm23(0, k)
                main_mm01(1, k - 2)
            main_mm01(1, KT - 2)
            main_mm01(1, KT - 1)
            lora01(0)
            lora23(0)
            main_close(0)

            # t1 second half + close
            q_t[1] = (q_t[1][0], open_pair("q3", "qm23_1"))
            for k in range(KT):
                main_mm23(1, k)
            lora01(1)
            lora23(1)
            main_close(1)

            # axT for tiles 2-3 / 4-7, interleaved with the tiles they feed
            ax_stage(xr23_sb, 2 * P, 2 * P, "qax23")

            def run_tile(t):
                e = 2 * (t % 2)
                q_t[t] = (open_pair(f"q{e}", f"qm01_{t}"),
                          open_pair(f"q{e + 1}", f"qm23_{t}"))
                for k in range(KT):
                    main_mm01(t, k)
                    main_mm23(t, k)
                lora01(t)
                lora23(t)
                main_close(t)

            run_tile(2)
            ax_stage(xr47_sb, 4 * P, 4 * P, "qax47")
            for t in range(3, TT - 1):
                run_tile(t)

            # last tile: close each bank-pair as soon as its half is done,
            # so the second half's matmuls overlap the first half's copy+DMA
            t = TT - 1
            e = 2 * (t % 2)
            q_t[t] = (open_pair(f"q{e}", f"qm01_{t}"), None)
            for k in range(KT):
                main_mm01(t, k)
            lora01(t)
            qa = q_t[t][0]
            ot = opool.tile([P, N], bf16, tag="o", name=f"o_{t}")
            nc.vector.tensor_copy(ot[:, 0:2 * CH], qa[:])
            nc.gpsimd.dma_start(out[t * P:(t + 1) * P, 0:2 * CH],
                                ot[:, 0:2 * CH])
            # c2 then c3 on separate psum ring generations so c3's matmuls
            # don't serialize behind c2's copy; each chunk's copy+DMA then
            # overlaps the next chunk's matmuls
            x_sb, toff = t_x(t)
            a_sl = axt_sb[:, t * P:(t + 1) * P]
            for c, qtag in ((2, f"q{e + 1}"), (3, f"q{e}")):
                qc = open_pair(qtag, f"qm_{t}_c{c}")
                for k in range(KT):
                    lhsT = x_sb[:, k, toff * P:(toff + 1) * P]
                    nc.tensor.matmul(qc[:, 0:CH], lhsT,
                                     w_sb[k][:, c * CH:(c + 1) * CH],
                                     start=(k == 0), stop=False)
                nc.tensor.matmul(qc[:, 0:CH], a_sl,
                                 bk_sb[:, c * CH:(c + 1) * CH],
                                 start=False, stop=True)
                nc.scalar.copy(ot[:, c * CH:(c + 1) * CH], qc[:, 0:CH])
                nc.gpsimd.dma_start(out[t * P:(t + 1) * P, c * CH:(c + 1) * CH],
                                    ot[:, c * CH:(c + 1) * CH])
            q_t.pop(t)

    nc.compile()
    return nc


def _prep_inputs(x, W_orig, A_kernel, B_kernel):
    import ml_dtypes

    bf16 = ml_dtypes.bfloat16
    x = np.asarray(x, dtype=np.float32)
    W_orig = np.asarray(W_orig, dtype=np.float32)
    A_kernel = np.asarray(A_kernel, dtype=np.float32)
    B_kernel = np.asarray(B_kernel, dtype=np.float32)

    w3 = np.ascontiguousarray(
        W_orig.reshape(KT, P, N).astype(bf16))             # [KT, P, N]
    akm = np.ascontiguousarray(
        (A_kernel * np.float32(SCALE)).reshape(KT, P, RANK)
        .transpose(1, 0, 2).reshape(P, KT * RANK)
        .astype(bf16))                                     # [P, KT*RANK]
    bkm = np.ascontiguousarray(
        B_kernel.reshape(RANK, N).astype(bf16))            # [RANK, N]

    x2d = x.reshape(TOK, H)
    in_maps = []
    for i in range(NCORES):
        xs = x2d[i * TPC:(i + 1) * TPC].T                  # [H, TPC]
        xs = xs.reshape(KT, P, TPC).transpose(1, 0, 2)     # [P, KT, TPC]
        xs = xs.astype(bf16)
        in_maps.append({
            "x0": np.ascontiguousarray(xs[:, :, 0:P]),
            "x1": np.ascontiguousarray(xs[:, :, P:2 * P]),
            "xr23": np.ascontiguousarray(xs[:, :, 2 * P:4 * P]),
            "xr47": np.ascontiguousarray(xs[:, :, 4 * P:]),
            "w": w3,
            "ak": akm,
            "bk": bkm,
        })
    return in_maps


def kernel(x, W_orig, A_kernel, B_kernel):
    from concourse.bass_utils import run_bass_kernel_spmd

    if "nc" not in _CACHE:
        _CACHE["nc"] = _build_program()
    nc = _CACHE["nc"]

    in_maps = _prep_inputs(x, W_orig, A_kernel, B_kernel)
    res = run_bass_kernel_spmd(nc, in_maps, list(range(NCORES)))
    parts = [np.asarray(res.results[i]["out"]) for i in range(NCORES)]
    full = np.concatenate(parts, axis=0).astype(np.float32)   # [TOK, N]
    return full.reshape(B, S, NH, HD)
